# revision 25
# baseline (speedup 1.0000x reference)
"""GCN layer kernel for Trainium2, 8 NeuronCores.

out = D^-1/2 (A + I) D^-1/2 (x @ W) + bias   with A built dense from edge_index
(scatter-set semantics => duplicate edges collapse, matching the reference).

Sharding: 1D node/row partition over 8 cores (hardcoded). Host-side sharding
dedups each core's edges and bit-packs them into fixed-size int16 index/value
lists addressing a flat fp8 adjacency canvas: partition p, fp8 element
j*1024+i of the canvas holds A[r0+i, j*128+p], stored as int16 words carrying
two adjacent rows (0x38 = fp8 1.0 in the low/high byte). The per-node bucket
counts (degrees) ship alongside, so no cross-core collective is needed. Each
core builds its canvas in SBUF via gpsimd local_scatter (2016-word windows to
fit the Q7 scratch), computes the full support = x @ W on the PE (fp16), fuses
the column-side deg^-1/2 scale into the PSUM->SBUF copy (z = deg^-1/2 *
support), then contracts out_T[d, i] = sum_j z[j, d] * A_T[j, i] with fp16 x
fp8 matmuls and fp32 PSUM accumulation, interleaved window-by-window with the
scatter stream. Finally it scales rows by its own deg^-1/2 and adds bias.
Host only shards/reorders inputs and transposes/concats the outputs.
"""

import sys

for _p in ("/opt/trn_rl_repo", "/root/.axon_site/_ro/trn_rl_repo"):
    if _p not in sys.path:
        sys.path.append(_p)

import numpy as np

import concourse.bacc as bacc
import concourse.bass as bass
import concourse.mybir as mybir
import concourse.tile as tile

# Problem shape (hardcoded per contract)
N = 8192
DIN = 128
DOUT = 128
P = 128
NCORES = 8
NSHARD = N // NCORES          # 1024 rows per core
JT = N // P                   # 64 contraction tiles
WT = NSHARD // 2              # canvas words per contraction tile (512)
CANVW = JT * WT               # flat canvas words per partition (32768)
WELEM = 2016                  # scatter window words (Q7 scratch is 65472 B =
                              # 32 B/word; 2016 leaves ~1 KB margin)
NW = (CANVW + WELEM - 1) // WELEM   # 17 windows (last one 512 words)
FP8_ONE = 0x38                # fp8 e4m3 1.0 bit pattern

BF16 = mybir.dt.bfloat16
F32 = mybir.dt.float32
FP16 = mybir.dt.float16
FP8 = mybir.dt.float8e4
I16 = mybir.dt.int16

_COMPILED = {}


def build_nc(nidxw: int, debug: bool = False):
    nc = bacc.Bacc("TRN2", target_bir_lowering=False, debug=debug,
                   enable_asserts=False, num_devices=NCORES)

    # I/O (xt_in = x transposed d-major: [din, jt, node] so each partition's
    # DMA read is one contiguous run)
    xt_in = nc.dram_tensor("xt_in", [DIN, JT, P], FP16, kind="ExternalInput")
    w = nc.dram_tensor("w", [DIN, DOUT], FP16, kind="ExternalInput")
    bias_in = nc.dram_tensor("bias_in", [DOUT, 1], F32, kind="ExternalInput")
    idx_in = nc.dram_tensor("idx_in", [P, NW, nidxw], I16,
                            kind="ExternalInput")
    val_in = nc.dram_tensor("val_in", [P, NW, nidxw], I16,
                            kind="ExternalInput")
    # Degree counts (bucket sizes incl. self loop): all nodes in (p, j) layout
    # for the column-side scale, own rows in row order for the row-side scale.
    dega_in = nc.dram_tensor("dega_in", [P, JT], F32, kind="ExternalInput")
    dego_in = nc.dram_tensor("dego_in", [NSHARD, 1], F32, kind="ExternalInput")
    out_t = nc.dram_tensor("out_t", [DOUT, NSHARD], F32, kind="ExternalOutput")

    with tile.TileContext(nc) as tc:
        with (
            tc.tile_pool(name="const", bufs=1) as cpool,
            tc.tile_pool(name="canv", bufs=1) as canvpool,
            tc.tile_pool(name="work", bufs=1) as wpool,
            tc.tile_pool(name="psA", bufs=2, space="PSUM") as psA,
            tc.tile_pool(name="psB", bufs=1, space="PSUM") as psB,
            tc.tile_pool(name="psO", bufs=1, space="PSUM") as psO,
        ):
            # ---------- constants / small loads ----------
            # tiny dummy scatter with no input deps: pays the ext-isa library
            # IRAM load (~4us) while the input DMAs are still in flight
            warm_idx = cpool.tile([16, 2], I16, tag="warm_idx")
            nc.gpsimd.memset(warm_idx[:, :], -1)
            warm_dst = cpool.tile([16, 2], FP16, tag="warm_dst")
            warm_dat = cpool.tile([16, 2], FP16, tag="warm_dat")
            nc.gpsimd.memset(warm_dat[:, :], 0.0)
            nc.gpsimd.local_scatter(
                out_ap=warm_dst[:, :], data_ap=warm_dat[:, :],
                idxs_ap=warm_idx[:, :], channels=16, num_elems=2, num_idxs=2)

            # scatter-window index/value lists, contiguous per partition:
            # critical path -- first half loaded before the bulk x transfer
            # so early scatter windows never queue behind it
            idx_sb = cpool.tile([P, NW, nidxw], I16, tag="idx_sb")
            val_sb = cpool.tile([P, NW, nidxw], I16, tag="val_sb")
            nc.sync.dma_start(out=idx_sb[:, 0:1, :], in_=idx_in[:, 0:1, :])
            nc.sync.dma_start(out=val_sb[:, 0:1, :], in_=val_in[:, 0:1, :])

            w_sb = cpool.tile([DIN, DOUT], FP16, tag="w_sb")
            nc.scalar.dma_start(out=w_sb[:, :], in_=w[:, :])

            # disall[p, j] = deg(node j*128+p) ^ -1/2 (column-side scale)
            disall = wpool.tile([P, JT], F32, tag="disall")
            nc.scalar.dma_start(out=disall[:, :], in_=dega_in[:, :])
            nc.vector.reciprocal(out=disall[:, :], in_=disall[:, :])
            nc.scalar.sqrt(out=disall[:, :], in_=disall[:, :])

            # ---------- z = deg^-1/2 * (x @ W), fused into PSUM->SBUF copy ----
            # x loaded in 8 chunked DMAs alternating issue engines so the
            # transfers land on distinct HWDGE queues and run concurrently
            # (one queue moves only ~50 GB/s; per-DMA issue also costs ~600ns
            # of engine time, so per-tile DMAs would serialize z production)
            xt_all = cpool.tile([DIN, JT, P], FP16, tag="xt_all")
            NXQ = 16
            for q in range(NXQ):
                qs, qe = q * (JT // NXQ), (q + 1) * (JT // NXQ)
                eng = nc.scalar if q % 2 == 0 else nc.sync
                eng.dma_start(out=xt_all[:, qs:qe, :],
                              in_=xt_in[:, qs:qe, :])
                if q == 3:
                    # mid-priority: windows 1-8 needed from ~14us in
                    nc.sync.dma_start(out=idx_sb[:, 1:9, :],
                                      in_=idx_in[:, 1:9, :])
                    nc.scalar.dma_start(out=val_sb[:, 1:9, :],
                                        in_=val_in[:, 1:9, :])
            # lowest priority: needed only from ~28us (windows 9+) / epilogue
            nc.sync.dma_start(out=idx_sb[:, 9:NW, :], in_=idx_in[:, 9:NW, :])
            nc.scalar.dma_start(out=val_sb[:, 9:NW, :], in_=val_in[:, 9:NW, :])
            bias_sb = cpool.tile([DOUT, 1], F32, tag="bias_sb")
            nc.scalar.dma_start(out=bias_sb[:, :], in_=bias_in[:, :])
            z_sb = cpool.tile([P, JT, DOUT], FP16, tag="z_sb")
            for j in range(JT):
                ps_s = psA.tile([P, P], F32, tag="ps_s")
                nc.tensor.matmul(out=ps_s[:, :], lhsT=xt_all[:, j, :],
                                 rhs=w_sb[:, :], start=True, stop=True)
                nc.vector.tensor_scalar_mul(out=z_sb[:, j, :],
                                            in0=ps_s[:, :],
                                            scalar1=disall[:, j:j + 1])

            # ---------- build packed fp8 canvas in SBUF via scatters ---------
            canv_sb = canvpool.tile([P, CANVW], I16, tag="cm")
            for wi in range(NW):
                lo = wi * WELEM
                we = min(WELEM, CANVW - lo)
                nc.gpsimd.local_scatter(
                    out_ap=canv_sb[:, lo:lo + we],
                    data_ap=val_sb[:, wi, :],
                    idxs_ap=idx_sb[:, wi, :],
                    channels=P, num_elems=we, num_idxs=nidxw)

            # row-side scale factors for this core's rows: rsqrt on a [1, 1024]
            # row, then broadcast across partitions with two rank-1 outer-
            # product matmuls (a 4KB DMA instead of a 512KB broadcast DMA that
            # would steal queue bandwidth from the x transfer)
            disrow = wpool.tile([1, NSHARD], F32, tag="disrow")
            nc.sync.dma_start(
                out=disrow[:, :],
                in_=dego_in.ap().rearrange("f one -> (one) f"))
            nc.vector.reciprocal(out=disrow[:, :], in_=disrow[:, :])
            nc.scalar.sqrt(out=disrow[:, :], in_=disrow[:, :])
            disrow_h = wpool.tile([1, NSHARD], FP16, tag="disrow_h")
            nc.vector.tensor_copy(out=disrow_h[:, :], in_=disrow[:, :])
            ones_col = wpool.tile([1, P], FP16, tag="ones_col")
            nc.vector.memset(ones_col[:, :], 1.0)
            disbig = wpool.tile([P, NSHARD], F32, tag="disbig")
            H = NSHARD // 2
            ps_b = psB.tile([P, NSHARD], F32, tag="ps_b")
            nc.tensor.matmul(out=ps_b[:, 0:H], lhsT=ones_col[:, :],
                             rhs=disrow_h[:, 0:H], start=True, stop=True)
            nc.tensor.matmul(out=ps_b[:, H:NSHARD], lhsT=ones_col[:, :],
                             rhs=disrow_h[:, H:NSHARD], start=True, stop=True)
            nc.vector.tensor_copy(out=disbig[:, 0:H], in_=ps_b[:, 0:H])
            nc.vector.tensor_copy(out=disbig[:, H:NSHARD],
                                  in_=ps_b[:, H:NSHARD])

            # ---------- main contraction out_T[d, i], interleaved w/ scatters -
            HW_ = WT // 2
            ps_o0 = psO.tile([P, H], F32, tag="ps_o0")
            ps_o1 = psO.tile([P, H], F32, tag="ps_o1")
            for j in range(JT):
                first = (j == 0)
                last = (j == JT - 1)
                base = j * WT
                nc.tensor.matmul(out=ps_o0[:, :], lhsT=z_sb[:, j, :],
                                 rhs=canv_sb[:, base:base + HW_].bitcast(FP8),
                                 start=first, stop=last)
                nc.tensor.matmul(out=ps_o1[:, :], lhsT=z_sb[:, j, :],
                                 rhs=canv_sb[:, base + HW_:base + WT]
                                 .bitcast(FP8),
                                 start=first, stop=last)

            # ---------- row scale + bias + store (4-chunk pipeline) ----------
            o_sb = wpool.tile([P, NSHARD], F32, tag="o_sb")
            Q = NSHARD // 4
            for k in range(4):
                lo, hi = k * Q, (k + 1) * Q
                ps = ps_o0 if k < 2 else ps_o1
                plo, phi = (lo, hi) if k < 2 else (lo - H, hi - H)
                nc.vector.tensor_tensor(out=o_sb[:, lo:hi],
                                        in0=ps[:, plo:phi],
                                        in1=disbig[:, lo:hi],
                                        op=mybir.AluOpType.mult)
                nc.scalar.activation(
                    out=o_sb[:, lo:hi], in_=o_sb[:, lo:hi],
                    func=mybir.ActivationFunctionType.Identity,
                    bias=bias_sb[:, 0:1], scale=1.0)
                eng = nc.sync if k % 2 == 0 else nc.scalar
                eng.dma_start(out=out_t[:, lo:hi], in_=o_sb[:, lo:hi])

    nc.compile()
    return nc


def shard_inputs(x, weight, bias, edge_index):
    """Host-side sharding: row-partition nodes over cores; dedup each core's
    edges, bit-pack adjacent rows into int16 fp8-pair words, bucket by scatter
    window, and count per-row entries (= node degrees incl. self loop)."""
    x = np.asarray(x, dtype=np.float32)
    weight = np.ascontiguousarray(np.asarray(weight, dtype=np.float16))
    bias = np.asarray(bias, dtype=np.float32).reshape(DOUT, 1)
    ei = np.asarray(edge_index, dtype=np.int64)
    rows, cols = ei[0], ei[1]

    # x transposed d-major to [din, jt, node] (replicated to every core)
    xt = np.ascontiguousarray(
        x.reshape(JT, P, DIN).transpose(2, 0, 1).astype(np.float16))

    # global degree = unique-edge count per row + 1 for the self loop
    m_all = rows != cols
    key_all = np.unique(rows[m_all] * N + cols[m_all])
    deg = 1.0 + np.bincount(key_all // N, minlength=N).astype(np.float32)
    # (p, j) layout: node g = j*128 + p
    dega = np.ascontiguousarray(deg.reshape(JT, P).T.astype(np.float32))

    core_packs = []
    nidxw = 2
    for c in range(NCORES):
        r0 = c * NSHARD
        m = (rows >= r0) & (rows < r0 + NSHARD) & (rows != cols)
        # unique (col, local_row) keys, duplicates collapsed; plus self loops
        key = np.unique(cols[m] * NSHARD + (rows[m] - r0))
        own = np.arange(r0, r0 + NSHARD, dtype=np.int64)
        key = np.concatenate([key, own * NSHARD + (own - r0)])
        g = key // NSHARD                      # global column
        i = key % NSHARD                       # local row
        p = g % P
        tw = (g // P) * WT + i // 2            # flat canvas word
        pat = np.where(i % 2 == 0, FP8_ONE, FP8_ONE << 8).astype(np.int64)
        # merge row-pairs: sum the lane patterns per (partition, word)
        pkey = p * CANVW + tw
        uk, inv = np.unique(pkey, return_inverse=True)
        uval = np.bincount(inv, weights=pat).astype(np.uint16)
        up = uk // CANVW
        utw = uk % CANVW
        wdw = utw // WELEM
        o = (utw - wdw * WELEM).astype(np.int16)
        # bucket by (partition, window); compute per-bucket positions
        bkey = (up * NW + wdw).astype(np.int64)
        order = np.argsort(bkey, kind="stable")
        bkey_s, o_s, v_s = bkey[order], o[order], uval[order]
        cnt = np.bincount(bkey_s, minlength=P * NW)
        nidxw = max(nidxw, int(cnt.max()))
        core_packs.append((bkey_s, o_s, v_s, cnt))
    nidxw = (nidxw + 1) // 2 * 2               # even

    in_maps = []
    for c in range(NCORES):
        bkey_s, o_s, v_s, cnt = core_packs[c]
        idx = np.full((P * NW, nidxw), -1, dtype=np.int16)
        val = np.zeros((P * NW, nidxw), dtype=np.uint16)
        pos = np.arange(len(bkey_s)) - np.repeat(np.cumsum(cnt) - cnt, cnt)
        idx[bkey_s, pos] = o_s
        val[bkey_s, pos] = v_s
        in_maps.append({
            "xt_in": xt,
            "w": weight,
            "bias_in": bias,
            "idx_in": np.ascontiguousarray(idx.reshape(P, NW, nidxw)),
            "val_in": np.ascontiguousarray(
                val.view(np.int16).reshape(P, NW, nidxw)),
            "dega_in": dega,
            "dego_in": deg[c * NSHARD:(c + 1) * NSHARD].reshape(NSHARD, 1),
        })
    return nidxw, in_maps


def _install_ntff_hook():
    """Provide antenv.axon_hooks if the image lacks it (profiling only)."""
    try:
        import antenv.axon_hooks  # noqa: F401
        return
    except ImportError:
        pass
    import types
    import antenv
    from trn_agent_boot.trn_boot import _ntff_profile_via_ctypes

    hook = _ntff_profile_via_ctypes("/opt/axon/libaxon_pjrt.so")
    mod = types.ModuleType("antenv.axon_hooks")
    mod._hook = hook
    mod.get_axon_ntff_profile_hook = lambda: mod._hook
    mod.set_axon_ntff_profile_hook = lambda h: setattr(mod, "_hook", h)
    sys.modules["antenv.axon_hooks"] = mod
    antenv.axon_hooks = mod


def kernel(x, weight, bias, edge_index, _trace=False):
    from concourse import bass_utils

    if _trace:
        _install_ntff_hook()

    nidxw, in_maps = shard_inputs(x, weight, bias, edge_index)
    if _COMPILED.get("nidxw") != nidxw:
        _COMPILED["nc"] = build_nc(nidxw)
        _COMPILED["nidxw"] = nidxw
    nc = _COMPILED["nc"]

    res = bass_utils.run_bass_kernel_spmd(
        nc, in_maps, core_ids=list(range(NCORES)), trace=_trace)
    if _trace:
        _COMPILED["last_results"] = res

    out = np.empty((N, DOUT), dtype=np.float32)
    for c in range(NCORES):
        out[c * NSHARD:(c + 1) * NSHARD, :] = res.results[c]["out_t"].T
    return out


# revision 27
# speedup vs baseline: 1.0390x; 1.0390x over previous
"""GCN layer kernel for Trainium2, 8 NeuronCores.

out = D^-1/2 (A + I) D^-1/2 (x @ W) + bias   with A built dense from edge_index
(scatter-set semantics => duplicate edges collapse, matching the reference).

Sharding: 1D node/row partition over 8 cores (hardcoded). Host-side sharding
dedups each core's edges and bit-packs them into fixed-size int16 index/value
lists addressing a flat fp8 adjacency canvas: partition p, fp8 element
j*1024+i of the canvas holds A[r0+i, j*128+p], stored as int16 words carrying
two adjacent rows (0x38 = fp8 1.0 in the low/high byte). The per-node bucket
counts (degrees) ship alongside, so no cross-core collective is needed. Each
core builds its canvas in SBUF via gpsimd local_scatter (2016-word windows to
fit the Q7 scratch), computes the full support = x @ W on the PE (fp16), fuses
the column-side deg^-1/2 scale into the PSUM->SBUF copy (z = deg^-1/2 *
support), then contracts out_T[d, i] = sum_j z[j, d] * A_T[j, i] with fp16 x
fp8 matmuls and fp32 PSUM accumulation, interleaved window-by-window with the
scatter stream. Finally it scales rows by its own deg^-1/2 and adds bias.
Host only shards/reorders inputs and transposes/concats the outputs.
"""

import sys

for _p in ("/opt/trn_rl_repo", "/root/.axon_site/_ro/trn_rl_repo"):
    if _p not in sys.path:
        sys.path.append(_p)

import numpy as np

import concourse.bacc as bacc
import concourse.bass as bass
import concourse.mybir as mybir
import concourse.tile as tile

# Problem shape (hardcoded per contract)
N = 8192
DIN = 128
DOUT = 128
P = 128
NCORES = 8
NSHARD = N // NCORES          # 1024 rows per core
JT = N // P                   # 64 contraction tiles
WT = NSHARD // 2              # canvas words per contraction tile (512)
CANVW = JT * WT               # flat canvas words per partition (32768)
WELEM = 2016                  # scatter window words (Q7 scratch is 65472 B =
                              # 32 B/word; 2016 leaves ~1 KB margin)
NW = (CANVW + WELEM - 1) // WELEM   # 17 windows (last one 512 words)
FP8_ONE = 0x38                # fp8 e4m3 1.0 bit pattern

BF16 = mybir.dt.bfloat16
F32 = mybir.dt.float32
FP16 = mybir.dt.float16
FP8 = mybir.dt.float8e4
I16 = mybir.dt.int16

_COMPILED = {}


def build_nc(nidxw: int, debug: bool = False):
    nc = bacc.Bacc("TRN2", target_bir_lowering=False, debug=debug,
                   enable_asserts=False, num_devices=NCORES)

    # I/O (xt_in = x transposed d-major: [din, jt, node] so each partition's
    # DMA read is one contiguous run)
    xt_in = nc.dram_tensor("xt_in", [DIN, JT, P], FP16, kind="ExternalInput")
    w = nc.dram_tensor("w", [DIN, DOUT], FP16, kind="ExternalInput")
    bias_in = nc.dram_tensor("bias_in", [DOUT, 1], F32, kind="ExternalInput")
    idx_in = nc.dram_tensor("idx_in", [P, NW, nidxw], I16,
                            kind="ExternalInput")
    val_in = nc.dram_tensor("val_in", [P, NW, nidxw], I16,
                            kind="ExternalInput")
    # Degree counts (bucket sizes incl. self loop): all nodes in (p, j) layout
    # for the column-side scale, own rows in row order for the row-side scale.
    dega_in = nc.dram_tensor("dega_in", [P, JT], F32, kind="ExternalInput")
    dego_in = nc.dram_tensor("dego_in", [NSHARD, 1], F32, kind="ExternalInput")
    out_t = nc.dram_tensor("out_t", [DOUT, NSHARD], F32, kind="ExternalOutput")

    with tile.TileContext(nc) as tc:
        with (
            tc.tile_pool(name="const", bufs=1) as cpool,
            tc.tile_pool(name="canv", bufs=1) as canvpool,
            tc.tile_pool(name="work", bufs=1) as wpool,
            tc.tile_pool(name="psA", bufs=2, space="PSUM") as psA,
            tc.tile_pool(name="psB", bufs=1, space="PSUM") as psB,
            tc.tile_pool(name="psO", bufs=1, space="PSUM") as psO,
        ):
            # ---------- constants / small loads ----------
            # tiny dummy scatter with no input deps: pays the ext-isa library
            # IRAM load (~4us) while the input DMAs are still in flight
            warm_idx = cpool.tile([16, 2], I16, tag="warm_idx")
            nc.gpsimd.memset(warm_idx[:, :], -1)
            warm_dst = cpool.tile([16, 2], FP16, tag="warm_dst")
            warm_dat = cpool.tile([16, 2], FP16, tag="warm_dat")
            nc.gpsimd.memset(warm_dat[:, :], 0.0)
            nc.gpsimd.local_scatter(
                out_ap=warm_dst[:, :], data_ap=warm_dat[:, :],
                idxs_ap=warm_idx[:, :], channels=16, num_elems=2, num_idxs=2)

            # scatter-window index/value lists, contiguous per partition:
            # critical path -- first half loaded before the bulk x transfer
            # so early scatter windows never queue behind it
            idx_sb = cpool.tile([P, NW, nidxw], I16, tag="idx_sb")
            val_sb = cpool.tile([P, NW, nidxw], I16, tag="val_sb")
            nc.sync.dma_start(out=idx_sb[:, 0:1, :], in_=idx_in[:, 0:1, :])
            nc.sync.dma_start(out=val_sb[:, 0:1, :], in_=val_in[:, 0:1, :])
            nc.sync.dma_start(out=idx_sb[:, 1:9, :], in_=idx_in[:, 1:9, :])
            nc.sync.dma_start(out=val_sb[:, 1:9, :], in_=val_in[:, 1:9, :])

            w_sb = cpool.tile([DIN, DOUT], FP16, tag="w_sb")
            nc.scalar.dma_start(out=w_sb[:, :], in_=w[:, :])
            bias_sb = cpool.tile([DOUT, 1], F32, tag="bias_sb")
            nc.scalar.dma_start(out=bias_sb[:, :], in_=bias_in[:, :])

            # disall[p, j] = deg(node j*128+p) ^ -1/2 (column-side scale)
            disall = wpool.tile([P, JT], F32, tag="disall")
            nc.scalar.dma_start(out=disall[:, :], in_=dega_in[:, :])
            nc.vector.reciprocal(out=disall[:, :], in_=disall[:, :])
            nc.scalar.sqrt(out=disall[:, :], in_=disall[:, :])

            # ---------- z = deg^-1/2 * (x @ W), fused into PSUM->SBUF copy ----
            # x loaded in 8 chunked DMAs alternating issue engines so the
            # transfers land on distinct HWDGE queues and run concurrently
            # (one queue moves only ~50 GB/s; per-DMA issue also costs ~600ns
            # of engine time, so per-tile DMAs would serialize z production)
            xt_all = cpool.tile([DIN, JT, P], FP16, tag="xt_all")
            for q in range(8):
                qs, qe = q * (JT // 8), (q + 1) * (JT // 8)
                eng = nc.scalar if q % 2 == 0 else nc.sync
                eng.dma_start(out=xt_all[:, qs:qe, :],
                              in_=xt_in[:, qs:qe, :])
            # remaining scatter windows: needed only from ~28us in, so these
            # queue behind the x transfer without hurting the scatter stream
            nc.sync.dma_start(out=idx_sb[:, 9:NW, :], in_=idx_in[:, 9:NW, :])
            nc.sync.dma_start(out=val_sb[:, 9:NW, :], in_=val_in[:, 9:NW, :])
            z_sb = cpool.tile([P, JT, DOUT], FP16, tag="z_sb")
            for j in range(JT):
                ps_s = psA.tile([P, P], F32, tag="ps_s")
                nc.tensor.matmul(out=ps_s[:, :], lhsT=xt_all[:, j, :],
                                 rhs=w_sb[:, :], start=True, stop=True)
                nc.vector.tensor_scalar_mul(out=z_sb[:, j, :],
                                            in0=ps_s[:, :],
                                            scalar1=disall[:, j:j + 1])

            # ---------- build packed fp8 canvas in SBUF via scatters ---------
            canv_sb = canvpool.tile([P, CANVW], I16, tag="cm")
            for wi in range(NW):
                lo = wi * WELEM
                we = min(WELEM, CANVW - lo)
                nc.gpsimd.local_scatter(
                    out_ap=canv_sb[:, lo:lo + we],
                    data_ap=val_sb[:, wi, :],
                    idxs_ap=idx_sb[:, wi, :],
                    channels=P, num_elems=we, num_idxs=nidxw)

            # row-side scale factors for this core's rows: rsqrt on a [1, 1024]
            # row, then broadcast across partitions with two rank-1 outer-
            # product matmuls (a 4KB DMA instead of a 512KB broadcast DMA that
            # would steal queue bandwidth from the x transfer)
            disrow = wpool.tile([1, NSHARD], F32, tag="disrow")
            nc.sync.dma_start(
                out=disrow[:, :],
                in_=dego_in.ap().rearrange("f one -> (one) f"))
            nc.vector.reciprocal(out=disrow[:, :], in_=disrow[:, :])
            nc.scalar.sqrt(out=disrow[:, :], in_=disrow[:, :])
            disrow_h = wpool.tile([1, NSHARD], FP16, tag="disrow_h")
            nc.vector.tensor_copy(out=disrow_h[:, :], in_=disrow[:, :])
            ones_col = wpool.tile([1, P], FP16, tag="ones_col")
            nc.vector.memset(ones_col[:, :], 1.0)
            disbig = wpool.tile([P, NSHARD], F32, tag="disbig")
            H = NSHARD // 2
            ps_b = psB.tile([P, NSHARD], F32, tag="ps_b")
            nc.tensor.matmul(out=ps_b[:, 0:H], lhsT=ones_col[:, :],
                             rhs=disrow_h[:, 0:H], start=True, stop=True)
            nc.tensor.matmul(out=ps_b[:, H:NSHARD], lhsT=ones_col[:, :],
                             rhs=disrow_h[:, H:NSHARD], start=True, stop=True)
            nc.vector.tensor_copy(out=disbig[:, 0:H], in_=ps_b[:, 0:H])
            nc.vector.tensor_copy(out=disbig[:, H:NSHARD],
                                  in_=ps_b[:, H:NSHARD])

            # ---------- main contraction out_T[d, i], interleaved w/ scatters -
            HW_ = WT // 2
            ps_o0 = psO.tile([P, H], F32, tag="ps_o0")
            ps_o1 = psO.tile([P, H], F32, tag="ps_o1")
            for j in range(JT):
                first = (j == 0)
                last = (j == JT - 1)
                base = j * WT
                nc.tensor.matmul(out=ps_o0[:, :], lhsT=z_sb[:, j, :],
                                 rhs=canv_sb[:, base:base + HW_].bitcast(FP8),
                                 start=first, stop=last)
                nc.tensor.matmul(out=ps_o1[:, :], lhsT=z_sb[:, j, :],
                                 rhs=canv_sb[:, base + HW_:base + WT]
                                 .bitcast(FP8),
                                 start=first, stop=last)

            # ---------- row scale + bias + store (4-chunk pipeline) ----------
            o_sb = wpool.tile([P, NSHARD], F32, tag="o_sb")
            Q = NSHARD // 4
            for k in range(4):
                lo, hi = k * Q, (k + 1) * Q
                ps = ps_o0 if k < 2 else ps_o1
                plo, phi = (lo, hi) if k < 2 else (lo - H, hi - H)
                nc.vector.tensor_tensor(out=o_sb[:, lo:hi],
                                        in0=ps[:, plo:phi],
                                        in1=disbig[:, lo:hi],
                                        op=mybir.AluOpType.mult)
                nc.scalar.activation(
                    out=o_sb[:, lo:hi], in_=o_sb[:, lo:hi],
                    func=mybir.ActivationFunctionType.Identity,
                    bias=bias_sb[:, 0:1], scale=1.0)
                eng = nc.sync if k % 2 == 0 else nc.scalar
                eng.dma_start(out=out_t[:, lo:hi], in_=o_sb[:, lo:hi])

    nc.compile()
    return nc


def shard_inputs(x, weight, bias, edge_index):
    """Host-side sharding: row-partition nodes over cores; dedup each core's
    edges, bit-pack adjacent rows into int16 fp8-pair words, bucket by scatter
    window, and count per-row entries (= node degrees incl. self loop)."""
    x = np.asarray(x, dtype=np.float32)
    weight = np.ascontiguousarray(np.asarray(weight, dtype=np.float16))
    bias = np.asarray(bias, dtype=np.float32).reshape(DOUT, 1)
    ei = np.asarray(edge_index, dtype=np.int64)
    rows, cols = ei[0], ei[1]

    # x transposed d-major to [din, jt, node] (replicated to every core)
    xt = np.ascontiguousarray(
        x.reshape(JT, P, DIN).transpose(2, 0, 1).astype(np.float16))

    # global degree = unique-edge count per row + 1 for the self loop
    m_all = rows != cols
    key_all = np.unique(rows[m_all] * N + cols[m_all])
    deg = 1.0 + np.bincount(key_all // N, minlength=N).astype(np.float32)
    # (p, j) layout: node g = j*128 + p
    dega = np.ascontiguousarray(deg.reshape(JT, P).T.astype(np.float32))

    core_packs = []
    nidxw = 2
    for c in range(NCORES):
        r0 = c * NSHARD
        m = (rows >= r0) & (rows < r0 + NSHARD) & (rows != cols)
        # unique (col, local_row) keys, duplicates collapsed; plus self loops
        key = np.unique(cols[m] * NSHARD + (rows[m] - r0))
        own = np.arange(r0, r0 + NSHARD, dtype=np.int64)
        key = np.concatenate([key, own * NSHARD + (own - r0)])
        g = key // NSHARD                      # global column
        i = key % NSHARD                       # local row
        p = g % P
        tw = (g // P) * WT + i // 2            # flat canvas word
        pat = np.where(i % 2 == 0, FP8_ONE, FP8_ONE << 8).astype(np.int64)
        # merge row-pairs: sum the lane patterns per (partition, word)
        pkey = p * CANVW + tw
        uk, inv = np.unique(pkey, return_inverse=True)
        uval = np.bincount(inv, weights=pat).astype(np.uint16)
        up = uk // CANVW
        utw = uk % CANVW
        wdw = utw // WELEM
        o = (utw - wdw * WELEM).astype(np.int16)
        # bucket by (partition, window); compute per-bucket positions
        bkey = (up * NW + wdw).astype(np.int64)
        order = np.argsort(bkey, kind="stable")
        bkey_s, o_s, v_s = bkey[order], o[order], uval[order]
        cnt = np.bincount(bkey_s, minlength=P * NW)
        nidxw = max(nidxw, int(cnt.max()))
        core_packs.append((bkey_s, o_s, v_s, cnt))
    nidxw = (nidxw + 1) // 2 * 2               # even

    in_maps = []
    for c in range(NCORES):
        bkey_s, o_s, v_s, cnt = core_packs[c]
        idx = np.full((P * NW, nidxw), -1, dtype=np.int16)
        val = np.zeros((P * NW, nidxw), dtype=np.uint16)
        pos = np.arange(len(bkey_s)) - np.repeat(np.cumsum(cnt) - cnt, cnt)
        idx[bkey_s, pos] = o_s
        val[bkey_s, pos] = v_s
        in_maps.append({
            "xt_in": xt,
            "w": weight,
            "bias_in": bias,
            "idx_in": np.ascontiguousarray(idx.reshape(P, NW, nidxw)),
            "val_in": np.ascontiguousarray(
                val.view(np.int16).reshape(P, NW, nidxw)),
            "dega_in": dega,
            "dego_in": deg[c * NSHARD:(c + 1) * NSHARD].reshape(NSHARD, 1),
        })
    return nidxw, in_maps


def _install_ntff_hook():
    """Provide antenv.axon_hooks if the image lacks it (profiling only)."""
    try:
        import antenv.axon_hooks  # noqa: F401
        return
    except ImportError:
        pass
    import types
    import antenv
    from trn_agent_boot.trn_boot import _ntff_profile_via_ctypes

    hook = _ntff_profile_via_ctypes("/opt/axon/libaxon_pjrt.so")
    mod = types.ModuleType("antenv.axon_hooks")
    mod._hook = hook
    mod.get_axon_ntff_profile_hook = lambda: mod._hook
    mod.set_axon_ntff_profile_hook = lambda h: setattr(mod, "_hook", h)
    sys.modules["antenv.axon_hooks"] = mod
    antenv.axon_hooks = mod


def kernel(x, weight, bias, edge_index, _trace=False):
    from concourse import bass_utils

    if _trace:
        _install_ntff_hook()

    nidxw, in_maps = shard_inputs(x, weight, bias, edge_index)
    if _COMPILED.get("nidxw") != nidxw:
        _COMPILED["nc"] = build_nc(nidxw)
        _COMPILED["nidxw"] = nidxw
    nc = _COMPILED["nc"]

    res = bass_utils.run_bass_kernel_spmd(
        nc, in_maps, core_ids=list(range(NCORES)), trace=_trace)
    if _trace:
        _COMPILED["last_results"] = res

    out = np.empty((N, DOUT), dtype=np.float32)
    for c in range(NCORES):
        out[c * NSHARD:(c + 1) * NSHARD, :] = res.results[c]["out_t"].T
    return out
